# revision 1
# baseline (speedup 1.0000x reference)
"""Trainium2 Bass kernel for nn_AggregateLevels (FPN detection post-process).

Takes full inputs (B=16), shards batch across 8 NeuronCores (2 per core),
runs one shared SPMD NEFF, gathers per-core outputs.

Outputs per the reference: (scores (16,261888,2), boxes (16,261888,4),
anchors (16,261888,4)). Anchors are input-independent constants and are
built host-side (replicated per level, per the data-parallel scheme).
"""

import math
import numpy as np

IH = IW = 1024
B = 16
NCORES = 8
BSH = B // NCORES  # batches per core
N_ANCHORS = 261888

# level meta: i -> (h, hw, Fc, nchunks, row_offset, ha)
_LEVELS = []
_off = 0
for _i in range(2, 7):
    _h = IH // (1 << _i)
    _hw = _h * _h
    _fc = min(_hw // 128, 256)
    _LEVELS.append(dict(i=_i, h=_h, hw=_hw, fc=_fc, nch=_hw // (128 * _fc),
                        off=_off, ha=float(IH / (1 << (6 - _i)))))
    _off += 3 * _hw
assert _off == N_ANCHORS


def _center_planes(lv):
    """Anchor center planes cy/cx for one level, flattened over (h, w)."""
    h = lv["h"]
    s = np.float32(2.0 ** lv["i"])
    c = (s * np.arange(h, dtype=np.float32) + np.float32(0.5) * (s - np.float32(1.0)))
    cy = np.broadcast_to(c[:, None], (h, h)).ravel()
    cx = np.broadcast_to(c[None, :], (h, h)).ravel()
    return np.ascontiguousarray(cy), np.ascontiguousarray(cx)


_anchors_cache = None


def _anchors_full():
    global _anchors_cache
    if _anchors_cache is None:
        parts = []
        for lv in _LEVELS:
            h, hw = lv["h"], lv["hw"]
            ha = np.float32(lv["ha"])
            cy, cx = _center_planes(lv)
            hs = np.array([ha, ha, 2 * ha], dtype=np.float32)
            ws = np.array([2 * ha, ha, ha], dtype=np.float32)
            a = np.empty((3, hw, 4), dtype=np.float32)
            a[:, :, 0] = cy[None, :]
            a[:, :, 1] = cx[None, :]
            a[:, :, 2] = hs[:, None]
            a[:, :, 3] = ws[:, None]
            parts.append(a.reshape(3 * hw, 4))
        one = np.concatenate(parts, axis=0)
        _anchors_cache = np.broadcast_to(one[None], (B, N_ANCHORS, 4))
    return _anchors_cache


_nc_cache = {}


def build_nc(loop_k=1):
    """Build the per-core Bass program (SPMD, identical on all cores)."""
    import concourse.bacc as bacc
    import concourse.mybir as mybir
    from concourse import tile
    from contextlib import ExitStack

    f32 = mybir.dt.float32
    nc = bacc.Bacc("TRN2", target_bir_lowering=False, debug=False)

    cs_h, bp_h, ay_h, ax_h = {}, {}, {}, {}
    for lv in _LEVELS:
        i, h = lv["i"], lv["h"]
        cs_h[i] = nc.dram_tensor(f"cs{i}", [BSH, 6, h, h], f32, kind="ExternalInput")
        bp_h[i] = nc.dram_tensor(f"bp{i}", [BSH, 12, h, h], f32, kind="ExternalInput")
        ay_h[i] = nc.dram_tensor(f"acy{i}", [lv["hw"]], f32, kind="ExternalInput")
        ax_h[i] = nc.dram_tensor(f"acx{i}", [lv["hw"]], f32, kind="ExternalInput")
    scores_h = nc.dram_tensor("scores", [BSH, N_ANCHORS, 2], f32, kind="ExternalOutput")
    boxes_h = nc.dram_tensor("boxes", [BSH, N_ANCHORS, 4], f32, kind="ExternalOutput")

    def body(ctx, tc):
        anch = ctx.enter_context(tc.tile_pool(name="anch", bufs=2))
        work = ctx.enter_context(tc.tile_pool(name="work", bufs=3))
        cpool = ctx.enter_context(tc.tile_pool(name="cpool", bufs=3))
        outp = ctx.enter_context(tc.tile_pool(name="outp", bufs=3))

        for lv in _LEVELS:
            i, hw, fc, off, ha = lv["i"], lv["hw"], lv["fc"], lv["off"], lv["ha"]
            ch_sz = 128 * fc
            hs = [ha, ha, 2.0 * ha]
            ws = [2.0 * ha, ha, ha]
            csr = cs_h[i][:].rearrange("b c h w -> b c (h w)")
            bpr = bp_h[i][:].rearrange("b c h w -> b c (h w)")
            for chunk in range(lv["nch"]):
                lo = chunk * ch_sz
                ay = anch.tile([128, fc], f32, tag="ay")
                nc.sync.dma_start(
                    ay[:], ay_h[i][lo:lo + ch_sz].rearrange("(q f) -> q f", q=128))
                ax = anch.tile([128, fc], f32, tag="ax")
                nc.sync.dma_start(
                    ax[:], ax_h[i][lo:lo + ch_sz].rearrange("(q f) -> q f", q=128))
                for b in range(BSH):
                    cs_t = work.tile([128, 6 * fc], f32, tag="cs")
                    nc.sync.dma_start(
                        cs_t[:],
                        csr[b][:, lo:lo + ch_sz].rearrange("c (q f) -> q c f", q=128))
                    bp_t = work.tile([128, 12 * fc], f32, tag="bp")
                    nc.sync.dma_start(
                        bp_t[:],
                        bpr[b][:, lo:lo + ch_sz].rearrange("c (q f) -> q c f", q=128))

                    # ---- scores: interleave channel pairs (a c f) -> (a f c)
                    sc_t = outp.tile([128, 6 * fc], f32, tag="sc")
                    nc.gpsimd.tensor_copy(
                        sc_t[:].rearrange("p (a f c) -> p a f c", a=3, c=2),
                        cs_t[:].rearrange("p (a c f) -> p a f c", a=3, c=2))
                    for a in range(3):
                        row = off + a * hw + lo
                        nc.sync.dma_start(
                            scores_h[b, row:row + ch_sz, :]
                            .rearrange("(q f) c -> q f c", q=128),
                            sc_t[:, a * 2 * fc:(a + 1) * 2 * fc])

                    # ---- boxes
                    e_t = work.tile([128, 6 * fc], f32, tag="e")
                    # exp of all dh/dw channels (4a+2, 4a+3) in one shot
                    nc.scalar.activation(
                        e_t[:],
                        bp_t[:].rearrange("p (a k f) -> p a k f", a=3, k=4)[:, :, 2:4, :],
                        mybir.ActivationFunctionType.Exp)
                    bx_t = outp.tile([128, 12 * fc], f32, tag="bx")
                    bx_v = bx_t[:].rearrange("p (a f k) -> p a f k", a=3, k=4)
                    for a in range(3):
                        c_t = cpool.tile([128, 2 * fc], f32, tag="c")
                        # centers: c = delta * size + anchor_center
                        nc.vector.scalar_tensor_tensor(
                            c_t[:, 0:fc], bp_t[:, (4 * a) * fc:(4 * a + 1) * fc],
                            hs[a], ay[:],
                            mybir.AluOpType.mult, mybir.AluOpType.add)
                        nc.vector.scalar_tensor_tensor(
                            c_t[:, fc:2 * fc], bp_t[:, (4 * a + 1) * fc:(4 * a + 2) * fc],
                            ws[a], ax[:],
                            mybir.AluOpType.mult, mybir.AluOpType.add)
                        # corners: out = exp(d) * (+-size/2) + center
                        for k, (j, sgn, sz) in enumerate(
                                [(0, -1.0, hs[a]), (1, -1.0, ws[a]),
                                 (0, 1.0, hs[a]), (1, 1.0, ws[a])]):
                            nc.vector.scalar_tensor_tensor(
                                bx_v[:, a, :, k],
                                e_t[:, (2 * a + j) * fc:(2 * a + j + 1) * fc],
                                sgn * sz * 0.5,
                                c_t[:, j * fc:(j + 1) * fc],
                                mybir.AluOpType.mult, mybir.AluOpType.add)
                        bxa = bx_t[:, a * 4 * fc:(a + 1) * 4 * fc]
                        nc.vector.tensor_scalar(
                            bxa, bxa, 0.0, float(IH),
                            mybir.AluOpType.max, mybir.AluOpType.min)
                        row = off + a * hw + lo
                        nc.sync.dma_start(
                            boxes_h[b, row:row + ch_sz, :]
                            .rearrange("(q f) k -> q f k", q=128),
                            bxa)

    with tile.TileContext(nc) as tc:
        with ExitStack() as ctx:
            if loop_k > 1:
                with tc.For_i(0, loop_k, 1):
                    body(ctx, tc)
            else:
                body(ctx, tc)
    nc.finalize()
    return nc


def _get_nc(loop_k=1):
    if loop_k not in _nc_cache:
        _nc_cache[loop_k] = build_nc(loop_k)
    return _nc_cache[loop_k]


def _in_maps(inputs):
    maps = []
    planes = {}
    for lv in _LEVELS:
        cy, cx = _center_planes(lv)
        planes[lv["i"]] = (cy, cx)
    for c in range(NCORES):
        m = {}
        for lv in _LEVELS:
            i = lv["i"]
            m[f"cs{i}"] = np.ascontiguousarray(inputs[f"cs{i}"][c * BSH:(c + 1) * BSH])
            m[f"bp{i}"] = np.ascontiguousarray(inputs[f"bp{i}"][c * BSH:(c + 1) * BSH])
            m[f"acy{i}"], m[f"acx{i}"] = planes[i]
        maps.append(m)
    return maps


def run_sharded(inputs, loop_k=1, **kw):
    """Run the SPMD kernel; returns (scores, boxes) full arrays + raw result."""
    from concourse import bass_utils
    nc = _get_nc(loop_k)
    res = bass_utils.run_bass_kernel_spmd(nc, _in_maps(inputs),
                                          core_ids=list(range(NCORES)), **kw)
    scores = np.concatenate([res.results[c]["scores"] for c in range(NCORES)], axis=0)
    boxes = np.concatenate([res.results[c]["boxes"] for c in range(NCORES)], axis=0)
    return scores, boxes, res


def kernel(**inputs):
    assert int(inputs.get("img_h", IH)) == IH and int(inputs.get("img_w", IW)) == IW
    scores, boxes, _ = run_sharded(inputs)
    return scores, boxes, _anchors_full()


# revision 13
# speedup vs baseline: 31.4347x; 31.4347x over previous
"""Trainium2 Bass kernel for nn_AggregateLevels (FPN detection post-process).

Takes full inputs (B=16), shards batch across 8 NeuronCores (2 per core),
runs one shared SPMD NEFF, gathers per-core outputs.

Outputs per the reference: (scores (16,261888,2), boxes (16,261888,4),
anchors (16,261888,4)). Anchors are input-independent constants and are
built host-side (replicated per level, per the data-parallel scheme).
"""

import math
import numpy as np

IH = IW = 1024
B = 16
NCORES = 8
BSH = B // NCORES  # batches per core
N_ANCHORS = 261888

# level meta: i -> (h, hw, Fc, nchunks, row_offset, ha)
_LEVELS = []
_off = 0
for _i in range(2, 7):
    _h = IH // (1 << _i)
    _hw = _h * _h
    _fc = min(_hw // 128, 256)
    _LEVELS.append(dict(i=_i, h=_h, hw=_hw, fc=_fc, nch=_hw // (128 * _fc),
                        off=_off, ha=float(IH / (1 << (6 - _i)))))
    _off += 3 * _hw
assert _off == N_ANCHORS


def _center_planes(lv):
    """Anchor center planes cy/cx for one level, flattened over (h, w)."""
    h = lv["h"]
    s = np.float32(2.0 ** lv["i"])
    c = (s * np.arange(h, dtype=np.float32) + np.float32(0.5) * (s - np.float32(1.0)))
    cy = np.broadcast_to(c[:, None], (h, h)).ravel()
    cx = np.broadcast_to(c[None, :], (h, h)).ravel()
    return np.ascontiguousarray(cy), np.ascontiguousarray(cx)


_anchors_cache = None


def _anchors_full():
    global _anchors_cache
    if _anchors_cache is None:
        parts = []
        for lv in _LEVELS:
            h, hw = lv["h"], lv["hw"]
            ha = np.float32(lv["ha"])
            cy, cx = _center_planes(lv)
            hs = np.array([ha, ha, 2 * ha], dtype=np.float32)
            ws = np.array([2 * ha, ha, ha], dtype=np.float32)
            a = np.empty((3, hw, 4), dtype=np.float32)
            a[:, :, 0] = cy[None, :]
            a[:, :, 1] = cx[None, :]
            a[:, :, 2] = hs[:, None]
            a[:, :, 3] = ws[:, None]
            parts.append(a.reshape(3 * hw, 4))
        one = np.concatenate(parts, axis=0)
        _anchors_cache = np.broadcast_to(one[None], (B, N_ANCHORS, 4))
    return _anchors_cache


_nc_cache = {}


def build_nc(loop_k=1, bufs=3, load_eng="scalar", order=(3, 4, 5, 6, 2)):
    """Build the per-core Bass program (SPMD, identical on all cores)."""
    import concourse.bacc as bacc
    import concourse.mybir as mybir
    from concourse import tile
    from contextlib import ExitStack

    f32 = mybir.dt.float32
    nc = bacc.Bacc("TRN2", target_bir_lowering=False, debug=False)

    cs_h, bp_h, an_h = {}, {}, {}
    for lv in _LEVELS:
        i, h = lv["i"], lv["h"]
        cs_h[i] = nc.dram_tensor(f"cs{i}", [BSH, 6, h, h], f32, kind="ExternalInput")
        bp_h[i] = nc.dram_tensor(f"bp{i}", [BSH, 12, h, h], f32, kind="ExternalInput")
        an_h[i] = nc.dram_tensor(f"anc{i}", [2, lv["hw"]], f32, kind="ExternalInput")
    scores_h = nc.dram_tensor("scores", [BSH, N_ANCHORS, 2], f32, kind="ExternalOutput")
    boxes_h = nc.dram_tensor("boxes", [BSH, N_ANCHORS, 4], f32, kind="ExternalOutput")

    def body(ctx, tc):
        ld = getattr(nc, load_eng)
        anch = ctx.enter_context(tc.tile_pool(name="anch", bufs=2))
        work = ctx.enter_context(tc.tile_pool(name="work", bufs=bufs))
        cpool = ctx.enter_context(tc.tile_pool(name="cpool", bufs=bufs))
        outp = ctx.enter_context(tc.tile_pool(name="outp", bufs=bufs))

        lv_by_i = {lv["i"]: lv for lv in _LEVELS}
        for lv in [lv_by_i[i] for i in order]:
            i, hw, fc, off, ha = lv["i"], lv["hw"], lv["fc"], lv["off"], lv["ha"]
            ch_sz = 128 * fc
            hs = [ha, ha, 2.0 * ha]
            ws = [2.0 * ha, ha, ha]
            csr = cs_h[i][:].rearrange("b c h w -> b c (h w)")
            bpr = bp_h[i][:].rearrange("b c h w -> b c (h w)")
            for chunk in range(lv["nch"]):
                lo = chunk * ch_sz
                an_t = anch.tile([128, 2 * fc], f32, tag="an")
                ld.dma_start(
                    an_t[:],
                    an_h[i][:, lo:lo + ch_sz].rearrange("j (q f) -> q j f", q=128))
                for b in range(BSH):
                    cs_t = work.tile([128, 6 * fc], f32, tag="cs")
                    ld.dma_start(
                        cs_t[:],
                        csr[b][:, lo:lo + ch_sz].rearrange("c (q f) -> q c f", q=128))
                    bp_t = work.tile([128, 12 * fc], f32, tag="bp")
                    ld.dma_start(
                        bp_t[:],
                        bpr[b][:, lo:lo + ch_sz].rearrange("c (q f) -> q c f", q=128))

                    # ---- scores: interleave channel pairs (a c f) -> (a f c)
                    sc_t = outp.tile([128, 6 * fc], f32, tag="sc")
                    nc.gpsimd.tensor_copy(
                        sc_t[:].rearrange("p (a f c) -> p a f c", a=3, c=2),
                        cs_t[:].rearrange("p (a c f) -> p a f c", a=3, c=2))
                    nc.sync.dma_start(
                        scores_h[b, off:off + 3 * hw, :]
                        .rearrange("(a r) c -> a r c", a=3)[:, lo:lo + ch_sz, :]
                        .rearrange("a (q f) c -> q a f c", q=128),
                        sc_t[:].rearrange("p (a f c) -> p a f c", a=3, c=2))

                    # ---- boxes
                    e_t = work.tile([128, 6 * fc], f32, tag="e")
                    # exp of all dh/dw channels (4a+2, 4a+3) in one shot
                    nc.scalar.activation(
                        e_t[:],
                        bp_t[:].rearrange("p (a k f) -> p a k f", a=3, k=4)[:, :, 2:4, :],
                        mybir.ActivationFunctionType.Exp)
                    bx_t = outp.tile([128, 12 * fc], f32, tag="bx")
                    bx_v = bx_t[:].rearrange("p (a f k) -> p a f k", a=3, k=4)
                    for a in range(3):
                        c_t = cpool.tile([128, 2 * fc], f32, tag="c")
                        # centers: c = delta * size + anchor_center
                        nc.vector.scalar_tensor_tensor(
                            c_t[:, 0:fc], bp_t[:, (4 * a) * fc:(4 * a + 1) * fc],
                            hs[a], an_t[:, 0:fc],
                            mybir.AluOpType.mult, mybir.AluOpType.add)
                        nc.vector.scalar_tensor_tensor(
                            c_t[:, fc:2 * fc], bp_t[:, (4 * a + 1) * fc:(4 * a + 2) * fc],
                            ws[a], an_t[:, fc:2 * fc],
                            mybir.AluOpType.mult, mybir.AluOpType.add)
                        # corners: out = exp(d) * (+-size/2) + center
                        for k, (j, sgn, sz) in enumerate(
                                [(0, -1.0, hs[a]), (1, -1.0, ws[a]),
                                 (0, 1.0, hs[a]), (1, 1.0, ws[a])]):
                            nc.vector.scalar_tensor_tensor(
                                bx_v[:, a, :, k],
                                e_t[:, (2 * a + j) * fc:(2 * a + j + 1) * fc],
                                sgn * sz * 0.5,
                                c_t[:, j * fc:(j + 1) * fc],
                                mybir.AluOpType.mult, mybir.AluOpType.add)
                        bxa = bx_t[:, a * 4 * fc:(a + 1) * 4 * fc]
                        nc.vector.tensor_scalar(
                            bxa, bxa, 0.0, float(IH),
                            mybir.AluOpType.max, mybir.AluOpType.min)
                    nc.sync.dma_start(
                        boxes_h[b, off:off + 3 * hw, :]
                        .rearrange("(a r) k -> a r k", a=3)[:, lo:lo + ch_sz, :]
                        .rearrange("a (q f) k -> q a f k", q=128),
                        bx_t[:].rearrange("p (a f k) -> p a f k", a=3, k=4))

    with tile.TileContext(nc) as tc:
        with ExitStack() as ctx:
            if loop_k > 1:
                hint = [mybir.EngineType.DVE, mybir.EngineType.SP,
                        mybir.EngineType.Activation, mybir.EngineType.Pool]
                with tc.For_i(0, loop_k, 1, hint_engines=hint):
                    body(ctx, tc)
            else:
                body(ctx, tc)
    nc.finalize()
    return nc


def _get_nc(loop_k=1):
    if loop_k not in _nc_cache:
        _nc_cache[loop_k] = build_nc(loop_k)
    return _nc_cache[loop_k]


def _in_maps(inputs):
    maps = []
    planes = {}
    for lv in _LEVELS:
        cy, cx = _center_planes(lv)
        planes[lv["i"]] = np.stack([cy, cx], axis=0)
    for c in range(NCORES):
        m = {}
        for lv in _LEVELS:
            i = lv["i"]
            m[f"cs{i}"] = np.ascontiguousarray(inputs[f"cs{i}"][c * BSH:(c + 1) * BSH])
            m[f"bp{i}"] = np.ascontiguousarray(inputs[f"bp{i}"][c * BSH:(c + 1) * BSH])
            m[f"anc{i}"] = planes[i]
        maps.append(m)
    return maps


def run_sharded(inputs, loop_k=1, **kw):
    """Run the SPMD kernel; returns (scores, boxes) full arrays + raw result."""
    from concourse import bass_utils
    nc = _get_nc(loop_k)
    res = bass_utils.run_bass_kernel_spmd(nc, _in_maps(inputs),
                                          core_ids=list(range(NCORES)), **kw)
    scores = np.concatenate([res.results[c]["scores"] for c in range(NCORES)], axis=0)
    boxes = np.concatenate([res.results[c]["boxes"] for c in range(NCORES)], axis=0)
    return scores, boxes, res


def kernel(**inputs):
    assert int(inputs.get("img_h", IH)) == IH and int(inputs.get("img_w", IW)) == IW
    scores, boxes, _ = run_sharded(inputs)
    return scores, boxes, _anchors_full()


# revision 40
# speedup vs baseline: 36.9265x; 1.1747x over previous
"""Trainium2 Bass kernel for nn_AggregateLevels (FPN detection post-process).

Takes full inputs (B=16), shards batch across 8 NeuronCores (2 per core),
runs one shared SPMD NEFF, gathers per-core outputs.

Outputs per the reference: (scores (16,261888,2), boxes (16,261888,4),
anchors (16,261888,4)). Anchors are input-independent constants and are
built host-side (replicated per level, per the data-parallel scheme).
"""

import math
import numpy as np

IH = IW = 1024
B = 16
NCORES = 8
BSH = B // NCORES  # batches per core
N_ANCHORS = 261888

# level meta: i -> (h, hw, Fc, nchunks, row_offset, ha)
def _mk_levels(fcmax=256):
    levels = []
    off = 0
    for i in range(2, 7):
        h = IH // (1 << i)
        hw = h * h
        fc = min(hw // 128, fcmax)
        levels.append(dict(i=i, h=h, hw=hw, fc=fc, nch=hw // (128 * fc),
                           off=off, ha=float(IH / (1 << (6 - i)))))
        off += 3 * hw
    assert off == N_ANCHORS
    return levels


_LEVELS = _mk_levels()


def _center_planes(lv):
    """Anchor center planes cy/cx for one level, flattened over (h, w)."""
    h = lv["h"]
    s = np.float32(2.0 ** lv["i"])
    c = (s * np.arange(h, dtype=np.float32) + np.float32(0.5) * (s - np.float32(1.0)))
    cy = np.broadcast_to(c[:, None], (h, h)).ravel()
    cx = np.broadcast_to(c[None, :], (h, h)).ravel()
    return np.ascontiguousarray(cy), np.ascontiguousarray(cx)


_anchors_cache = None


def _anchors_full():
    global _anchors_cache
    if _anchors_cache is None:
        parts = []
        for lv in _LEVELS:
            h, hw = lv["h"], lv["hw"]
            ha = np.float32(lv["ha"])
            cy, cx = _center_planes(lv)
            hs = np.array([ha, ha, 2 * ha], dtype=np.float32)
            ws = np.array([2 * ha, ha, ha], dtype=np.float32)
            a = np.empty((3, hw, 4), dtype=np.float32)
            a[:, :, 0] = cy[None, :]
            a[:, :, 1] = cx[None, :]
            a[:, :, 2] = hs[:, None]
            a[:, :, 3] = ws[:, None]
            parts.append(a.reshape(3 * hw, 4))
        one = np.concatenate(parts, axis=0)
        _anchors_cache = np.broadcast_to(one[None], (B, N_ANCHORS, 4))
    return _anchors_cache


_nc_cache = {}


def build_nc(loop_k=1, bufs=4, load_eng="scalar", order=(3, 4, 5, 6, 2),
             sc_eng="vector", clip_eng="vector", staggered=True,
             store_eng="sync", fcmax=256, skip=()):
    """Build the per-core Bass program (SPMD, identical on all cores)."""
    import concourse.bacc as bacc
    import concourse.mybir as mybir
    from concourse import tile
    from contextlib import ExitStack

    f32 = mybir.dt.float32
    nc = bacc.Bacc("TRN2", target_bir_lowering=False, debug=False)

    cs_h, bp_h, an_h = {}, {}, {}
    for lv in _LEVELS:
        i, h = lv["i"], lv["h"]
        cs_h[i] = nc.dram_tensor(f"cs{i}", [BSH, 6, h, h], f32, kind="ExternalInput")
        bp_h[i] = nc.dram_tensor(f"bp{i}", [BSH, 12, h, h], f32, kind="ExternalInput")
        an_h[i] = nc.dram_tensor(f"anc{i}", [2, lv["hw"]], f32, kind="ExternalInput")
    scores_h = nc.dram_tensor("scores", [BSH, N_ANCHORS, 2], f32, kind="ExternalOutput")
    boxes_h = nc.dram_tensor("boxes", [BSH, N_ANCHORS, 4], f32, kind="ExternalOutput")

    def body(ctx, tc):
        ld = getattr(nc, load_eng)
        st_sc = getattr(nc, "scalar" if store_eng == "split" else store_eng)
        st_bx = getattr(nc, "sync" if store_eng == "split" else store_eng)
        anch = ctx.enter_context(tc.tile_pool(name="anch", bufs=2))
        work = ctx.enter_context(tc.tile_pool(name="work", bufs=bufs))
        cpool = ctx.enter_context(tc.tile_pool(name="cpool", bufs=bufs))
        outp = ctx.enter_context(tc.tile_pool(name="outp", bufs=bufs))

        lv_by_i = {lv["i"]: lv for lv in _mk_levels(fcmax)}
        # expand `order` (levels, possibly repeated) into (level, [chunks]):
        # chunks of a level are spread across its occurrences in order.
        occ = {}
        for i in order:
            occ[i] = occ.get(i, 0) + 1
        cursor = {i: 0 for i in occ}
        seen = {i: 0 for i in occ}
        worklist = []
        for i in order:
            lv = lv_by_i[i]
            seen[i] += 1
            n_here = lv["nch"] - cursor[i] if seen[i] == occ[i] else max(
                1, lv["nch"] // occ[i])
            chunks = list(range(cursor[i], cursor[i] + n_here))
            cursor[i] += n_here
            if chunks:
                worklist.append((lv, chunks))
        for lv, chunks in worklist:
            i, hw, fc, off, ha = lv["i"], lv["hw"], lv["fc"], lv["off"], lv["ha"]
            ch_sz = 128 * fc
            hs = [ha, ha, 2.0 * ha]
            ws = [2.0 * ha, ha, ha]
            csr = cs_h[i][:].rearrange("b c h w -> b c (h w)")
            bpr = bp_h[i][:].rearrange("b c h w -> b c (h w)")
            for chunk in chunks:
                lo = chunk * ch_sz
                an_t = anch.tile([128, 2 * fc], f32, tag="an")
                if "loads" in skip:
                    nc.vector.memset(an_t[:, 0:1], 0.0)
                else:
                    ld.dma_start(
                        an_t[:],
                        an_h[i][:, lo:lo + ch_sz].rearrange("j (q f) -> q j f", q=128))
                for b in range(BSH):
                    cs_t = work.tile([128, 6 * fc], f32, tag="cs")
                    bp_t = work.tile([128, 12 * fc], f32, tag="bp")
                    if "loads" in skip:
                        nc.vector.memset(cs_t[:, 0:1], 0.0)
                        nc.vector.memset(bp_t[:, 0:1], 0.0)
                    else:
                        ld.dma_start(
                            cs_t[:],
                            csr[b][:, lo:lo + ch_sz].rearrange("c (q f) -> q c f", q=128))
                        ld.dma_start(
                            bp_t[:],
                            bpr[b][:, lo:lo + ch_sz].rearrange("c (q f) -> q c f", q=128))

                    # ---- scores: interleave channel pairs (a c f) -> (a f c)
                    sc_t = outp.tile([128, 6 * fc], f32, tag="sc")
                    if "sc_dense" in skip:
                        getattr(nc, sc_eng).tensor_copy(sc_t[:], cs_t[:])
                    elif "scores" in skip:
                        nc.vector.memset(sc_t[:, 0:1], 0.0)
                    elif sc_eng == "scalar":
                        nc.scalar.copy(
                            sc_t[:].rearrange("p (a f c) -> p a f c", a=3, c=2),
                            cs_t[:].rearrange("p (a c f) -> p a f c", a=3, c=2))
                    else:
                        getattr(nc, sc_eng).tensor_copy(
                            sc_t[:].rearrange("p (a f c) -> p a f c", a=3, c=2),
                            cs_t[:].rearrange("p (a c f) -> p a f c", a=3, c=2))
                    if "stores" not in skip:
                        if "scsplit" in skip and i in (2, 3):
                            for a in range(3):
                                row = off + a * hw + lo
                                st_sc.dma_start(
                                    scores_h[b, row:row + ch_sz, :]
                                    .rearrange("(q f) c -> q f c", q=128),
                                    sc_t[:, a * 2 * fc:(a + 1) * 2 * fc])
                        else:
                            st_sc.dma_start(
                                scores_h[b, off:off + 3 * hw, :]
                                .rearrange("(a r) c -> a r c", a=3)[:, lo:lo + ch_sz, :]
                                .rearrange("a (q f) c -> q a f c", q=128),
                                sc_t[:].rearrange("p (a f c) -> p a f c", a=3, c=2))

                    # ---- boxes
                    bx_t = outp.tile([128, 12 * fc], f32, tag="bx")
                    bx_v = bx_t[:].rearrange("p (a f k) -> p a f k", a=3, k=4)
                    if "exp" in skip:
                        e_t = None
                        nc.vector.memset(bx_t[:, 0:1], 0.0)
                    else:
                        e_t = work.tile([128, 6 * fc], f32, tag="e")
                        # exp of all dh/dw channels (4a+2, 4a+3) in one shot
                        nc.scalar.activation(
                            e_t[:],
                            bp_t[:].rearrange("p (a k f) -> p a k f", a=3, k=4)[:, :, 2:4, :],
                            mybir.ActivationFunctionType.Exp)
                    for a in range(3):
                        c_t = (None if "centers" in skip
                               else cpool.tile([128, 2 * fc], f32, tag="c"))
                        if c_t is not None:
                            # centers: c = delta * size + anchor_center
                            nc.vector.scalar_tensor_tensor(
                                c_t[:, 0:fc], bp_t[:, (4 * a) * fc:(4 * a + 1) * fc],
                                hs[a], an_t[:, 0:fc],
                                mybir.AluOpType.mult, mybir.AluOpType.add)
                            nc.vector.scalar_tensor_tensor(
                                c_t[:, fc:2 * fc], bp_t[:, (4 * a + 1) * fc:(4 * a + 2) * fc],
                                ws[a], an_t[:, fc:2 * fc],
                                mybir.AluOpType.mult, mybir.AluOpType.add)
                        if "combine_dense" in skip:
                            # timing ablation: same element count, dense writes
                            for k, (j, sgn, sz) in enumerate(
                                    [(0, -1.0, hs[a]), (1, -1.0, ws[a]),
                                     (0, 1.0, hs[a]), (1, 1.0, ws[a])]):
                                nc.vector.scalar_tensor_tensor(
                                    bx_t[:, (4 * a + k) * fc:(4 * a + k + 1) * fc],
                                    e_t[:, (2 * a + j) * fc:(2 * a + j + 1) * fc],
                                    sgn * sz * 0.5,
                                    c_t[:, j * fc:(j + 1) * fc],
                                    mybir.AluOpType.mult, mybir.AluOpType.add)
                        elif "combine" not in skip:
                            # corners: out = exp(d) * (+-size/2) + center
                            for k, (j, sgn, sz) in enumerate(
                                    [(0, -1.0, hs[a]), (1, -1.0, ws[a]),
                                     (0, 1.0, hs[a]), (1, 1.0, ws[a])]):
                                nc.vector.scalar_tensor_tensor(
                                    bx_v[:, a, :, k],
                                    e_t[:, (2 * a + j) * fc:(2 * a + j + 1) * fc],
                                    sgn * sz * 0.5,
                                    c_t[:, j * fc:(j + 1) * fc],
                                    mybir.AluOpType.mult, mybir.AluOpType.add)
                        if "clip" not in skip:
                            bxa = bx_t[:, a * 4 * fc:(a + 1) * 4 * fc]
                            getattr(nc, clip_eng).tensor_scalar(
                                bxa, bxa, 0.0, float(IH),
                                mybir.AluOpType.max, mybir.AluOpType.min)
                    if "stores" not in skip:
                        if "bxsplit" in skip and i in (2, 3):
                            for a in range(3):
                                row = off + a * hw + lo
                                st_bx.dma_start(
                                    boxes_h[b, row:row + ch_sz, :]
                                    .rearrange("(q f) k -> q f k", q=128),
                                    bx_t[:, a * 4 * fc:(a + 1) * 4 * fc])
                        else:
                            st_bx.dma_start(
                                boxes_h[b, off:off + 3 * hw, :]
                                .rearrange("(a r) k -> a r k", a=3)[:, lo:lo + ch_sz, :]
                                .rearrange("a (q f) k -> q a f k", q=128),
                                bx_t[:].rearrange("p (a f k) -> p a f k", a=3, k=4))

    with tile.TileContext(nc) as tc:
        with ExitStack() as ctx:
            if loop_k > 1:
                hint = [mybir.EngineType.DVE, mybir.EngineType.SP,
                        mybir.EngineType.Activation, mybir.EngineType.Pool]
                with tc.For_i(0, loop_k, 1, hint_engines=hint,
                              staggered_reset=staggered):
                    body(ctx, tc)
            else:
                body(ctx, tc)
    nc.finalize()
    return nc


def _get_nc(loop_k=1, **kw):
    kw = {k: (tuple(v) if isinstance(v, list) else v) for k, v in kw.items()}
    key = (loop_k, tuple(sorted(kw.items())))
    if key not in _nc_cache:
        _nc_cache[key] = build_nc(loop_k, **kw)
    return _nc_cache[key]


def _in_maps(inputs):
    maps = []
    planes = {}
    for lv in _LEVELS:
        cy, cx = _center_planes(lv)
        planes[lv["i"]] = np.stack([cy, cx], axis=0)
    for c in range(NCORES):
        m = {}
        for lv in _LEVELS:
            i = lv["i"]
            m[f"cs{i}"] = np.ascontiguousarray(inputs[f"cs{i}"][c * BSH:(c + 1) * BSH])
            m[f"bp{i}"] = np.ascontiguousarray(inputs[f"bp{i}"][c * BSH:(c + 1) * BSH])
            m[f"anc{i}"] = planes[i]
        maps.append(m)
    return maps


def run_sharded(inputs, loop_k=1, build_kw=None, **kw):
    """Run the SPMD kernel; returns (scores, boxes) full arrays + raw result."""
    from concourse import bass_utils
    nc = _get_nc(loop_k, **(build_kw or {}))
    res = bass_utils.run_bass_kernel_spmd(nc, _in_maps(inputs),
                                          core_ids=list(range(NCORES)), **kw)
    scores = np.concatenate([res.results[c]["scores"] for c in range(NCORES)], axis=0)
    boxes = np.concatenate([res.results[c]["boxes"] for c in range(NCORES)], axis=0)
    return scores, boxes, res


def kernel(**inputs):
    assert int(inputs.get("img_h", IH)) == IH and int(inputs.get("img_w", IW)) == IW
    scores, boxes, _ = run_sharded(inputs)
    return scores, boxes, _anchors_full()
